# revision 8
# baseline (speedup 1.0000x reference)
"""Trainium2 8-core tensor-parallel sparse-attention kernel (Bass/Tile).

Reference computation (SQ=2048, B=1, H=2048, NH=16, HD=128):
    x = hidden[:,0,:] @ svd_token
    w = qkv_w @ svd_token
    mixed = x @ w.T + qkv_b -> split per head into q,k,v (head h owns cols h*384+[0:384))
    q,k rotated by svd_qk[h]; v rotated by svd_vlin[h]
    scores = q @ k.T / sqrt(128), causal mask, softmax
    ctx = probs @ v_rot
    tsr[h] = svd_vlin[h].T @ dense_w[h]  (stacked -> [2048, 2048])
    out = ctx @ tsr + dense_b

Sharding (Megatron-style TP over heads, 2 heads/core, 8 cores):
    stage A: x computed seq-sharded, AllGather
    stage B: w/mixed computed head-sharded (transposed layouts)
    stage C: per-head attention with causal tile skipping; softmax as raw
             exp (scores are O(15), no overflow) + normalization after PV
    stage D: tsr computed head-sharded, AllGather; ctx^T AllToAll so each
             core owns all heads for its seq block; local out-projection.
Host only shards inputs / concatenates the 8 output row-blocks.
"""
import math

import numpy as np

import concourse.bass as bass
import concourse.mybir as mybir
import concourse.bacc as bacc
import concourse.tile as tile
from concourse import bass_utils

N_CORES = 8
SQ = 2048
H = 2048
NH = 16
HD = 128
HPC = NH // N_CORES          # heads per core = 2
QKVR = HPC * 3 * HD          # qkv rows per core = 768
SEQB = SQ // N_CORES         # seq block per core = 256
KT = H // 128                # k tiles over hidden = 16
F32 = mybir.dt.float32
F32R = mybir.dt.float32r
NEG = -30000.0


def r(ap):
    """View an f32 AP as float32r for full-rate PE matmuls."""
    return ap.bitcast(F32R)


def build(causal=True):
    nc = bacc.Bacc("TRN2", target_bir_lowering=False, debug=False,
                   num_devices=N_CORES)

    hT = nc.dram_tensor("hT", [H, SEQB], F32, kind="ExternalInput")
    qwT = nc.dram_tensor("qwT", [H, QKVR], F32, kind="ExternalInput")
    qbT = nc.dram_tensor("qbT", [128, QKVR // 128], F32, kind="ExternalInput")
    stok = nc.dram_tensor("stok", [H, H], F32, kind="ExternalInput")
    sqk = nc.dram_tensor("sqk", [HPC, HD, HD], F32, kind="ExternalInput")
    svl = nc.dram_tensor("svl", [HPC, HD, HD], F32, kind="ExternalInput")
    dw = nc.dram_tensor("dw", [HPC, HD, H], F32, kind="ExternalInput")
    dbB = nc.dram_tensor("dbB", [1, H], F32, kind="ExternalInput")
    out = nc.dram_tensor("out", [SEQB, H], F32, kind="ExternalOutput")

    ones_np = np.ones((128, 128), np.float32)
    ones_dram = nc.inline_tensor(ones_np, name="ones_const")
    tb_np = np.where(
        np.arange(128)[:, None] > np.arange(896)[None, :] - 384, NEG, 0.0
    ).astype(np.float32)
    tb_dram = nc.inline_tensor(tb_np, name="triband_const")

    rg = [list(range(N_CORES))]

    with tile.TileContext(nc) as tc:
        with (
            nc.allow_low_precision(reason="f32r rounding for full-rate PE"),
            tc.tile_pool(name="pers", bufs=1) as pers,
            tc.tile_pool(name="dram", bufs=1, space="DRAM") as dram,
        ):
            # ---- persistent SBUF tensors ----
            ones_sb = pers.tile([128, 128], F32)
            tb_sb = pers.tile([128, 896], F32)
            nc.sync.dma_start(r(ones_sb[:]), r(ones_dram[:]))
            nc.sync.dma_start(tb_sb[:], tb_dram[:])

            qb_sb = pers.tile([128, QKVR // 128], F32)
            nc.sync.dma_start(qb_sb[:], qbT[:])
            sqk_sb = pers.tile([128, HPC * HD], F32)
            svl_sb = pers.tile([128, HPC * HD], F32)
            for hl in range(HPC):
                nc.sync.dma_start(r(sqk_sb[:, hl * HD:(hl + 1) * HD]), r(sqk[hl]))
                nc.sync.dma_start(r(svl_sb[:, hl * HD:(hl + 1) * HD]), r(svl[hl]))
            db_sb = pers.tile([1, H], F32)
            nc.sync.dma_start(r(db_sb[:]), r(dbB[:]))

            # ---- stage 0: tsr shard + AllGather (overlaps everything) ----
            tsr_in = dram.tile([HPC * HD, H], F32)
            tsr_g = dram.tile([NH * HD, H], F32, addr_space="Shared")
            with (
                tc.tile_pool(name="s0", bufs=2) as s0,
                tc.tile_pool(name="p0", bufs=2, space="PSUM") as p0,
            ):
                dw_sb = s0.tile([128, HPC * H], F32, tag="dw")
                for hl in range(HPC):
                    nc.sync.dma_start(r(dw_sb[:, hl * H:(hl + 1) * H]), r(dw[hl]))
                for hl in range(HPC):
                    tsr_sb = s0.tile([128, H], F32, tag="tsr")
                    for n in range(4):
                        tp = p0.tile([128, 512], F32, tag="tsrp")
                        nc.tensor.matmul(
                            tp[:], r(svl_sb[:, hl * HD:(hl + 1) * HD]),
                            r(dw_sb[:, hl * H + n * 512: hl * H + (n + 1) * 512]),
                            start=True, stop=True)
                        nc.vector.tensor_copy(tsr_sb[:, n * 512:(n + 1) * 512], tp[:])
                    nc.sync.dma_start(tsr_in[hl * HD:(hl + 1) * HD, :], tsr_sb[:])
            nc.gpsimd.collective_compute(
                "AllGather", mybir.AluOpType.bypass, replica_groups=rg,
                ins=[tsr_in[:].opt()], outs=[tsr_g[:].opt()])

            # ---- stage A+B1: x_c (seq-shard of x^T) and w_shard^T ----
            # x^T_c[j, s] = sum_k svd_token[k, j] * hidden^T[k, s]
            # wT[j, r] = sum_k svd_token[k, j] * qkv_w^T[k, r]
            xc_in = dram.tile([H, SEQB], F32)
            x_g = dram.tile([N_CORES * H, SEQB], F32, addr_space="Shared")
            wT_sb = pers.tile([128, KT * QKVR], F32)   # 16 j-tiles of [128, 768]
            with (
                tc.tile_pool(name="sA", bufs=2) as sA,
                tc.tile_pool(name="pA", bufs=2, space="PSUM") as pA,
            ):
                hT_sb = sA.tile([128, KT * SEQB], F32, tag="hT", bufs=1)
                qwT_sb = sA.tile([128, KT * QKVR], F32, tag="qwT", bufs=1)
                for k in range(KT):
                    nc.sync.dma_start(r(hT_sb[:, k * SEQB:(k + 1) * SEQB]),
                                      r(hT[k * 128:(k + 1) * 128, :]))
                    nc.sync.dma_start(r(qwT_sb[:, k * QKVR:(k + 1) * QKVR]),
                                      r(qwT[k * 128:(k + 1) * 128, :]))
                for j in range(KT):
                    stc = sA.tile([128, KT * 128], F32, tag="stc", bufs=3)
                    for k in range(KT):
                        nc.sync.dma_start(
                            r(stc[:, k * 128:(k + 1) * 128]),
                            r(stok[k * 128:(k + 1) * 128, j * 128:(j + 1) * 128]))
                    xp = pA.tile([128, SEQB], F32, tag="xp")
                    for k in range(KT):
                        nc.tensor.matmul(
                            xp[:], r(stc[:, k * 128:(k + 1) * 128]),
                            r(hT_sb[:, k * SEQB:(k + 1) * SEQB]),
                            start=(k == 0), stop=(k == KT - 1))
                    xs = sA.tile([128, SEQB], F32, tag="xs")
                    nc.vector.tensor_copy(xs[:], xp[:])
                    nc.sync.dma_start(xc_in[j * 128:(j + 1) * 128, :], xs[:])
                    for o0, nw in ((0, 512), (512, 256)):
                        wp = pA.tile([128, 512], F32, tag="wp")
                        for k in range(KT):
                            nc.tensor.matmul(
                                wp[:, :nw], r(stc[:, k * 128:(k + 1) * 128]),
                                r(qwT_sb[:, k * QKVR + o0: k * QKVR + o0 + nw]),
                                start=(k == 0), stop=(k == KT - 1))
                        nc.vector.tensor_copy(
                            r(wT_sb[:, j * QKVR + o0: j * QKVR + o0 + nw]), wp[:, :nw])
            nc.gpsimd.collective_compute(
                "AllGather", mybir.AluOpType.bypass, replica_groups=rg,
                ins=[xc_in[:].opt()], outs=[x_g[:].opt()])

            # ---- stage B2: mixed^T = w_shard @ x^T  [768, 2048] ----
            MT = QKVR // 128  # 6 m-tiles
            mixT = pers.tile([128, MT * SQ], F32)   # 6 tiles of [128, 2048]
            with (
                tc.tile_pool(name="sB", bufs=3) as sB,
                tc.tile_pool(name="pB", bufs=2, space="PSUM") as pB,
            ):
                for rb in range(N_CORES):
                    xg_sb = sB.tile([128, KT * SEQB], F32, tag="xg")
                    for k in range(KT):
                        nc.sync.dma_start(
                            r(xg_sb[:, k * SEQB:(k + 1) * SEQB]),
                            r(x_g[rb * H + k * 128: rb * H + (k + 1) * 128, :]))
                    for mt in range(MT):
                        mp = pB.tile([128, SEQB], F32, tag="mp")
                        for k in range(KT):
                            nc.tensor.matmul(
                                mp[:],
                                r(wT_sb[:, k * QKVR + mt * 128: k * QKVR + (mt + 1) * 128]),
                                r(xg_sb[:, k * SEQB:(k + 1) * SEQB]),
                                start=(k == 0), stop=(k == KT - 1))
                        nc.vector.tensor_scalar_add(
                            r(mixT[:, mt * SQ + rb * SEQB: mt * SQ + (rb + 1) * SEQB]),
                            mp[:], qb_sb[:, mt:mt + 1])

            # ---- stage C: rotations + attention per head ----
            ctx_in = dram.tile([N_CORES, HPC * HD, SEQB], F32)
            ctx_a = dram.tile([N_CORES, HPC * HD, SEQB], F32)
            SCALE = 1.0 / math.sqrt(HD)
            with (
                tc.tile_pool(name="sC", bufs=1) as sC,
                tc.tile_pool(name="pC", bufs=2, space="PSUM") as pC,
                tc.tile_pool(name="sD", bufs=2) as sD,
            ):
                for hl in range(HPC):
                    qrow, krow, vrow = hl * 3, hl * 3 + 1, hl * 3 + 2
                    qrotT = sC.tile([128, SQ], F32, tag="qrot", bufs=2)
                    krotT = sC.tile([128, SQ], F32, tag="krot", bufs=2)
                    vrot = sC.tile([128, SQ], F32, tag="vrot", bufs=2)
                    for sc in range(4):
                        rp = pC.tile([128, 512], F32, tag="rotp")
                        nc.tensor.matmul(
                            rp[:], r(sqk_sb[:, hl * HD:(hl + 1) * HD]),
                            r(mixT[:, qrow * SQ + sc * 512: qrow * SQ + (sc + 1) * 512]),
                            start=True, stop=True)
                        nc.scalar.activation(
                            r(qrotT[:, sc * 512:(sc + 1) * 512]), rp[:],
                            mybir.ActivationFunctionType.Copy, scale=SCALE)
                        rp2 = pC.tile([128, 512], F32, tag="rotp")
                        nc.tensor.matmul(
                            rp2[:], r(sqk_sb[:, hl * HD:(hl + 1) * HD]),
                            r(mixT[:, krow * SQ + sc * 512: krow * SQ + (sc + 1) * 512]),
                            start=True, stop=True)
                        nc.vector.tensor_copy(r(krotT[:, sc * 512:(sc + 1) * 512]), rp2[:])
                    for st in range(KT):
                        vp = pC.tile([128, 128], F32, tag="rotp")
                        nc.tensor.matmul(
                            vp[:],
                            r(mixT[:, vrow * SQ + st * 128: vrow * SQ + (st + 1) * 128]),
                            r(svl_sb[:, hl * HD:(hl + 1) * HD]),
                            start=True, stop=True)
                        nc.vector.tensor_copy(r(vrot[:, st * 128:(st + 1) * 128]), vp[:])

                    ctxT_sb = sC.tile([128, SQ], F32, tag="ctxT", bufs=2)
                    for rb in range(4):
                        ncb = 4 * (rb + 1) if causal else KT
                        ctp = pC.tile([128, 512], F32, tag="ctp")
                        lp = pC.tile([1, 512], F32, tag="lp", bufs=1)
                        for cb in range(ncb):
                            sp = pC.tile([128, 512], F32, tag="sp")
                            nc.tensor.matmul(
                                sp[:], r(krotT[:, cb * 128:(cb + 1) * 128]),
                                r(qrotT[:, rb * 512:(rb + 1) * 512]),
                                start=True, stop=True)
                            if causal and cb >= 4 * rb:
                                d = cb * 128 - rb * 512
                                o = 384 - d
                                nc.vector.tensor_tensor(
                                    sp[:], sp[:], tb_sb[:, o:o + 512],
                                    mybir.AluOpType.add)
                            pT = sD.tile([128, 512], F32, tag="pT", bufs=3)
                            nc.scalar.activation(
                                r(pT[:]), sp[:], mybir.ActivationFunctionType.Exp)
                            nc.tensor.matmul(
                                ctp[:], r(vrot[:, cb * 128:(cb + 1) * 128]), r(pT[:]),
                                start=(cb == 0), stop=(cb == ncb - 1))
                            nc.tensor.matmul(
                                lp[:], r(ones_sb[:, 0:1]), r(pT[:]),
                                start=(cb == 0), stop=(cb == ncb - 1))
                        linv = sD.tile([1, 512], F32, tag="linv")
                        nc.vector.reciprocal(r(linv[:]), lp[:])
                        lbp = pC.tile([128, 512], F32, tag="lbp", bufs=1)
                        nc.tensor.matmul(lbp[:], r(ones_sb[0:1, :]), r(linv[:]),
                                         start=True, stop=True)
                        lb_sb = sD.tile([128, 512], F32, tag="lb")
                        nc.vector.tensor_copy(lb_sb[:], lbp[:])
                        nc.vector.tensor_tensor(
                            ctxT_sb[:, rb * 512:(rb + 1) * 512], ctp[:], lb_sb[:],
                            mybir.AluOpType.mult)
                    # scatter ctxT to A2A shards: shard b gets seq cols b*256..
                    for b in range(N_CORES):
                        nc.sync.dma_start(
                            ctx_in[b, hl * HD:(hl + 1) * HD, :],
                            ctxT_sb[:, b * SEQB:(b + 1) * SEQB])
            nc.gpsimd.collective_compute(
                "AllToAll", mybir.AluOpType.bypass, replica_groups=rg,
                ins=[ctx_in[:].opt()], outs=[ctx_a[:].opt()])

            # ---- stage E: out rows = ctx[myblock,:] @ tsr + dense_b ----
            with (
                tc.tile_pool(name="sE", bufs=2) as sE,
                tc.tile_pool(name="pE", bufs=2, space="PSUM") as pE,
            ):
                bb_sb = sE.tile([128, H], F32, tag="bb", bufs=1)
                for n in range(4):
                    bp = pE.tile([128, 512], F32, tag="bp")
                    nc.tensor.matmul(bp[:], r(ones_sb[0:1, :]),
                                     r(db_sb[:, n * 512:(n + 1) * 512]),
                                     start=True, stop=True)
                    nc.vector.tensor_copy(bb_sb[:, n * 512:(n + 1) * 512], bp[:])
                ctxa_sb = sE.tile([128, NH * HD * SEQB // 128], F32, tag="ctxa", bufs=1)
                # k-tile kt (global (h,e) block) = ctx_a[kt//2, (kt%2)*128..., :]
                for kt in range(KT):
                    nc.sync.dma_start(
                        r(ctxa_sb[:, kt * SEQB:(kt + 1) * SEQB]),
                        r(ctx_a[kt // HPC, (kt % HPC) * HD:((kt % HPC) + 1) * HD, :]))
                for n in range(4):
                    tsr_sb2 = sE.tile([128, KT * 512], F32, tag="tsrs", bufs=2)
                    for kt in range(KT):
                        nc.sync.dma_start(
                            r(tsr_sb2[:, kt * 512:(kt + 1) * 512]),
                            r(tsr_g[kt * 128:(kt + 1) * 128, n * 512:(n + 1) * 512]))
                    for mt in range(2):
                        op = pE.tile([128, 512], F32, tag="op")
                        for kt in range(KT):
                            nc.tensor.matmul(
                                op[:],
                                r(ctxa_sb[:, kt * SEQB + mt * 128: kt * SEQB + (mt + 1) * 128]),
                                r(tsr_sb2[:, kt * 512:(kt + 1) * 512]),
                                start=(kt == 0), stop=(kt == KT - 1))
                        os_ = sE.tile([128, 512], F32, tag="os")
                        nc.vector.tensor_tensor(
                            os_[:], op[:], bb_sb[:, n * 512:(n + 1) * 512],
                            mybir.AluOpType.add)
                        nc.sync.dma_start(
                            out[mt * 128:(mt + 1) * 128, n * 512:(n + 1) * 512],
                            os_[:])
    nc.compile()
    return nc


_CAUSAL_MASK = None


def _is_causal(mask):
    global _CAUSAL_MASK
    m = np.asarray(mask).reshape(SQ, SQ)
    if _CAUSAL_MASK is None:
        _CAUSAL_MASK = np.triu(np.ones((SQ, SQ), dtype=bool), k=1)
    return np.array_equal(m, _CAUSAL_MASK)


def make_in_maps(inputs):
    hidden_states = np.asarray(inputs["hidden_states"], np.float32)
    qkv_w = np.asarray(inputs["qkv_w"], np.float32)
    qkv_b = np.asarray(inputs["qkv_b"], np.float32)
    svd_token = np.ascontiguousarray(np.asarray(inputs["svd_token"], np.float32))
    svd_qk = np.asarray(inputs["svd_qk"], np.float32)
    svd_vlin = np.asarray(inputs["svd_vlin"], np.float32)
    dense_w = np.asarray(inputs["dense_w"], np.float32)
    dense_b = np.asarray(inputs["dense_b"], np.float32)

    hTf = np.ascontiguousarray(hidden_states[:, 0, :].T)        # [H, SQ]
    qwTf = np.ascontiguousarray(qkv_w.T)                        # [H, 3H]
    in_maps = []
    for c in range(N_CORES):
        h0 = c * HPC
        rows = slice(c * QKVR, (c + 1) * QKVR)
        in_maps.append({
            "hT": np.ascontiguousarray(hTf[:, c * SEQB:(c + 1) * SEQB]),
            "qwT": np.ascontiguousarray(qwTf[:, rows]),
            "qbT": np.ascontiguousarray(qkv_b[rows].reshape(QKVR // 128, 128).T),
            "stok": svd_token,
            "sqk": np.ascontiguousarray(svd_qk[h0:h0 + HPC]),
            "svl": np.ascontiguousarray(svd_vlin[h0:h0 + HPC]),
            "dw": np.ascontiguousarray(dense_w[h0:h0 + HPC]),
            "dbB": np.ascontiguousarray(dense_b.reshape(1, H)),
        })
    return in_maps


def kernel(hidden_states, attention_mask, qkv_w, qkv_b, svd_token,
           svd_qk, svd_vlin, dense_w, dense_b):
    causal = _is_causal(attention_mask)
    if not causal:
        assert not np.asarray(attention_mask).any(), \
            "kernel supports causal or empty attention_mask"

    nc = build(causal=causal)
    in_maps = make_in_maps({
        "hidden_states": hidden_states, "qkv_w": qkv_w, "qkv_b": qkv_b,
        "svd_token": svd_token, "svd_qk": svd_qk, "svd_vlin": svd_vlin,
        "dense_w": dense_w, "dense_b": dense_b,
    })
    res = bass_utils.run_bass_kernel_spmd(
        nc, in_maps, core_ids=list(range(N_CORES)), trace=False)
    full = np.concatenate([res.results[c]["out"] for c in range(N_CORES)], axis=0)
    return full.reshape(SQ, 1, H)
